# revision 1
# baseline (speedup 1.0000x reference)
"""WaveNet-style gated residual block (AdvancedSkipResidualBlock) on 8 TRN2 NeuronCores.

Strategy: data-parallel over batch B=8 -> one batch element per NeuronCore.
Per core, the whole block is 7 GEMMs of [512,512] weights x [512, T] activations:
  xc   = x + Wc @ cond + bc                      (1 GEMM,  K=512)
  f    = tanh(W_f0 @ xc(t-2) + W_f1 @ xc(t) + bf)  (2 GEMMs, dilated causal conv K=2, d=2)
  g    = sigmoid(W_g0 @ xc(t-2) + W_g1 @ xc(t) + bg)
  h    = f * g
  res  = Wr @ h + br + xc                        (1 GEMM)
  skip = Ws @ h + bs                             (1 GEMM)
Matmuls run in bf16 (fp32 PSUM accumulation); elementwise/bias/activation fused
onto ScalarE (tanh/sigmoid/identity, one LUT table set) and VectorE
(scalar_tensor_tensor fuses bias-add + residual-add in one op).
Time axis is processed in chunks (128/128/256 ramp-up, then 512s) with a
2-column halo for the dilated conv; a 3-stage software pipeline
(cond -> conv/gate -> res/skip) keeps the tensor engine at its issue rate.
Measured: ~223us NEFF exec at full clock (bf16 matmul roofline ~191us/core).
"""

import os
import sys
from contextlib import ExitStack

import numpy as np

try:
    import concourse.bass as bass  # noqa: F401
except ImportError:  # pragma: no cover
    sys.path.insert(0, "/opt/trn_rl_repo")
    import concourse.bass as bass  # noqa: F401

import ml_dtypes  # noqa: E402
import concourse.tile as tile  # noqa: E402
from concourse import bacc, mybir  # noqa: E402
from concourse.bass_utils import run_bass_kernel_spmd  # noqa: E402

B, C, T = 8, 512, 4096
P, G = 128, 4          # SBUF partitions, channel groups (C = G*P)
CH = 512               # steady-state time-chunk width
# Small leading chunks let the PE start while weights/x are still streaming
# in; small trailing chunks shorten the post-matmul drain. Steady-state
# chunks are 512 (one PSUM bank of fp32).
CWS = [128, 128, 256] + [512] * 7
assert sum(CWS) == T
LOS = [0]
for _w in CWS:
    LOS.append(LOS[-1] + _w)
NCH = len(CWS)
DIL = 2                # conv dilation (kernel_size=2 -> taps at t-2 and t)

BF16 = mybir.dt.bfloat16
F32 = mybir.dt.float32
AF = mybir.ActivationFunctionType
ALU = mybir.AluOpType

_CACHE: dict = {}


def _build():
    nc = bacc.Bacc("TRN2", target_bir_lowering=False, debug=False, num_devices=B)

    def din(name, shape, dt):
        return nc.dram_tensor(name, shape, dt, kind="ExternalInput").ap()

    def dout(name, shape, dt):
        return nc.dram_tensor(name, shape, dt, kind="ExternalOutput").ap()

    x_d = din("x", [C, T], BF16)
    c_d = din("cond", [C, T], BF16)
    wc_d = din("wc", [P, G, G, P], BF16)
    wf_d = din("wf", [P, 2, G, G, P], BF16)
    wg_d = din("wg", [P, 2, G, G, P], BF16)
    wr_d = din("wr", [P, G, G, P], BF16)
    ws_d = din("ws", [P, G, G, P], BF16)
    bias_d = din("bias", [P, 5 * G], F32)
    r_d = dout("res", [C, T], F32)
    s_d = dout("skip", [C, T], F32)

    x_r = x_d.rearrange("(g p) t -> p g t", p=P)
    c_r = c_d.rearrange("(g p) t -> p g t", p=P)
    r_r = r_d.rearrange("(g p) t -> p g t", p=P)
    s_r = s_d.rearrange("(g p) t -> p g t", p=P)

    with tile.TileContext(nc) as tc, ExitStack() as ctx:
        const = ctx.enter_context(tc.tile_pool(name="const", bufs=1))
        xin = ctx.enter_context(tc.tile_pool(name="xin", bufs=4))
        cin = ctx.enter_context(tc.tile_pool(name="cin", bufs=4))
        xcp = ctx.enter_context(tc.tile_pool(name="xcp", bufs=4))
        fgp = ctx.enter_context(tc.tile_pool(name="fgp", bufs=4))
        hp = ctx.enter_context(tc.tile_pool(name="hp", bufs=3))
        rop = ctx.enter_context(tc.tile_pool(name="rop", bufs=6))
        sop = ctx.enter_context(tc.tile_pool(name="sop", bufs=6))
        psum = ctx.enter_context(tc.tile_pool(name="psum", bufs=8, space="PSUM"))

        # Startup choreography. The first PE work (cond matmuls of chunk 0)
        # needs cond(0) + wc; the fg matmuls soon after need x0 then wf/wg.
        # Sync (HWDGE) queue: c0, wc, x0, biases in that order; the big
        # conv weights stream concurrently on the GpSimd (SWDGE) queue so
        # neither queue serializes the other's critical bytes. Output
        # stores ride the Scalar (HWDGE) queue.
        from concourse.tile_rust import add_dep_helper

        ct0 = cin.tile([P, G, CWS[0]], BF16, tag="c")
        nc.sync.dma_start(ct0[:], c_r[:, :, 0:CWS[0]])
        wc_sb = const.tile([P, G, G, P], BF16)
        i_wc = nc.sync.dma_start(wc_sb[:], wc_d)
        xt0 = xin.tile([P, G, CWS[0]], BF16, tag="x")
        nc.gpsimd.dma_start(xt0[:], x_r[:, :, 0:CWS[0]])
        # Conv weights, split per tap so the first f-matmuls only wait on
        # tap 0. The first conv-weight DMA is held behind wc's completion
        # so the SWDGE stream doesn't steal HBM bandwidth from the bytes
        # that gate the very first matmul.
        wf_tap = []
        wg_tap = []
        for tap in range(2):
            t = const.tile([P, G, G, P], BF16, tag=f"wf{tap}")
            i = nc.gpsimd.dma_start(t[:], wf_d[:, tap])
            if tap == 0:
                add_dep_helper(i.ins, i_wc.ins, reason="prioritize wc bytes at startup")
            wf_tap.append(t)
        for tap in range(2):
            t = const.tile([P, G, G, P], BF16, tag=f"wg{tap}")
            nc.gpsimd.dma_start(t[:], wg_d[:, tap])
            wg_tap.append(t)
        wr_sb = const.tile([P, G, G, P], BF16)
        nc.gpsimd.dma_start(wr_sb[:], wr_d)
        ws_sb = const.tile([P, G, G, P], BF16)
        nc.gpsimd.dma_start(ws_sb[:], ws_d)
        pre_c = {0: ct0}
        pre_x = {0: xt0}
        # all 5 biases in one DMA: five separate triggers cost ~3us of sync
        # engine time ahead of the chunk-1 loads
        b_all = const.tile([P, 5 * G], F32)
        nc.sync.dma_start(b_all[:], bias_d)
        _bidx = {"bc": 0, "bf": 1, "bg": 2, "br": 3, "bs": 4}

        def bias_ap(name, m):
            return b_all[:, _bidx[name] * G + m:_bidx[name] * G + m + 1]

        xc_t: dict = {}
        h_t: dict = {}
        for it in range(NCH + 2):
            c0, c1, c2 = it, it - 1, it - 2

            # ---- stage 1: condition injection (chunk c0) ----
            if c0 < NCH:
                w, lo = CWS[c0], LOS[c0]
                if c0 in pre_c:
                    ct, xt = pre_c.pop(c0), pre_x.pop(c0)
                else:
                    ct = cin.tile([P, G, w], BF16, tag="c")
                    nc.sync.dma_start(ct[:], c_r[:, :, lo:lo + w])
                    xt = xin.tile([P, G, w], BF16, tag="x")
                    nc.sync.dma_start(xt[:], x_r[:, :, lo:lo + w])
                xc = xcp.tile([P, G, w + DIL], BF16, tag="xc")
                if c0 == 0:
                    nc.vector.memset(xc[:, :, 0:DIL], 0.0)
                else:
                    pw = CWS[c0 - 1]
                    nc.vector.tensor_copy(xc[:, :, 0:DIL], xc_t[c0 - 1][:, :, pw:pw + DIL])
                for m in range(G):
                    ps = psum.tile([P, w], F32, space="PSUM", tag="ps")
                    for k in range(G):
                        nc.tensor.matmul(ps, wc_sb[:, k, m, :], ct[:, k, :],
                                         start=(k == 0), stop=(k == G - 1))
                    # xc[m] = (Wc@cond + bc) + x, cast to bf16
                    nc.vector.scalar_tensor_tensor(
                        xc[:, m, DIL:w + DIL], ps, bias_ap("bc", m),
                        xt[:, m, :], ALU.add, ALU.add)
                xc_t[c0] = xc

            # ---- stage 2: dilated conv + gated activation (chunk c1) ----
            if 0 <= c1 < NCH:
                w = CWS[c1]
                xc = xc_t[c1]
                h = hp.tile([P, G, w], BF16, tag="h")
                for m in range(G):
                    pf = psum.tile([P, w], F32, space="PSUM", tag="ps")
                    for k in range(G):
                        nc.tensor.matmul(pf, wf_tap[0][:, k, m, :], xc[:, k, 0:w],
                                         start=(k == 0), stop=False)
                    for k in range(G):
                        nc.tensor.matmul(pf, wf_tap[1][:, k, m, :], xc[:, k, DIL:w + DIL],
                                         start=False, stop=(k == G - 1))
                    fsb = fgp.tile([P, w], BF16, tag="f")
                    nc.scalar.activation(fsb[:], pf, AF.Tanh, bias=bias_ap("bf", m))
                    pg = psum.tile([P, w], F32, space="PSUM", tag="ps")
                    for k in range(G):
                        nc.tensor.matmul(pg, wg_tap[0][:, k, m, :], xc[:, k, 0:w],
                                         start=(k == 0), stop=False)
                    for k in range(G):
                        nc.tensor.matmul(pg, wg_tap[1][:, k, m, :], xc[:, k, DIL:w + DIL],
                                         start=False, stop=(k == G - 1))
                    gsb = fgp.tile([P, w], BF16, tag="g")
                    nc.scalar.activation(gsb[:], pg, AF.Sigmoid, bias=bias_ap("bg", m))
                    nc.vector.tensor_mul(h[:, m, :], fsb[:], gsb[:])
                h_t[c1] = h

            # ---- stage 3: residual + skip projections (chunk c2) ----
            if 0 <= c2 < NCH:
                w, lo = CWS[c2], LOS[c2]
                h = h_t.pop(c2)
                xc = xc_t.pop(c2)
                for m in range(G):
                    pr = psum.tile([P, w], F32, space="PSUM", tag="ps")
                    for k in range(G):
                        nc.tensor.matmul(pr, wr_sb[:, k, m, :], h[:, k, :],
                                         start=(k == 0), stop=(k == G - 1))
                    # res[m] = (Wr@h + br) + xc; per-m store so the tail
                    # chunk's output DMA starts before all 4 m-tiles finish
                    rt = rop.tile([P, w], F32, tag="r")
                    nc.vector.scalar_tensor_tensor(
                        rt[:], pr, bias_ap("br", m),
                        xc[:, m, DIL:w + DIL], ALU.add, ALU.add)
                    nc.scalar.dma_start(r_r[:, m, lo:lo + w], rt[:])
                    pk = psum.tile([P, w], F32, space="PSUM", tag="ps")
                    for k in range(G):
                        nc.tensor.matmul(pk, ws_sb[:, k, m, :], h[:, k, :],
                                         start=(k == 0), stop=(k == G - 1))
                    st = sop.tile([P, w], F32, tag="s")
                    nc.scalar.activation(st[:], pk, AF.Identity,
                                         bias=bias_ap("bs", m))
                    nc.scalar.dma_start(s_r[:, m, lo:lo + w], st[:])

    nc.compile()
    return nc


def _get_nc():
    if "nc" not in _CACHE:
        _CACHE["nc"] = _build()
    return _CACHE["nc"]


def _wT1(w):
    # [Cout, Cin, 1] -> lhsT layout [P(cin%P), G(cin//P), G(cout//P), P(cout%P)]
    return np.ascontiguousarray(
        np.asarray(w)[:, :, 0].T.reshape(G, P, G, P).transpose(1, 0, 2, 3)
        .astype(ml_dtypes.bfloat16))


def _wT2(w):
    # [Cout, Cin, 2] -> [P, tap, G(cin//P), G(cout//P), P]
    taps = [np.asarray(w)[:, :, t].T.reshape(G, P, G, P).transpose(1, 0, 2, 3)
            for t in range(2)]
    return np.ascontiguousarray(np.stack(taps, axis=1).astype(ml_dtypes.bfloat16))


def _bias(b):
    return np.ascontiguousarray(np.asarray(b).reshape(G, P).T.astype(np.float32))


def kernel(x, condition, wf, bf, wg, bg, wr, br, ws, bs, wc, bc):
    nc = _get_nc()
    x_bf = np.asarray(x).astype(ml_dtypes.bfloat16)
    cond_bf = np.asarray(condition).astype(ml_dtypes.bfloat16)
    shared = {
        "wc": _wT1(wc), "wf": _wT2(wf), "wg": _wT2(wg),
        "wr": _wT1(wr), "ws": _wT1(ws),
        "bias": np.ascontiguousarray(np.concatenate(
            [_bias(b) for b in (bc, bf, bg, br, bs)], axis=1)),
    }
    in_maps = [
        {"x": np.ascontiguousarray(x_bf[i]), "cond": np.ascontiguousarray(cond_bf[i]),
         **shared}
        for i in range(B)
    ]
    res = run_bass_kernel_spmd(
        nc, in_maps, list(range(B)),
        trace=bool(os.environ.get("CC_KERNEL_TRACE")))
    _CACHE["last_results"] = res
    residual = np.stack([res.results[i]["res"] for i in range(B)])
    skip = np.stack([res.results[i]["skip"] for i in range(B)])
    return residual, skip



# revision 7
# speedup vs baseline: 1.0548x; 1.0548x over previous
"""WaveNet-style gated residual block (AdvancedSkipResidualBlock) on 8 TRN2 NeuronCores.

Strategy: data-parallel over batch B=8 -> one batch element per NeuronCore.
Per core the block is 7 GEMMs of [512,512] x [512,T]; the two dilated causal
convs (K=2, d=2) are restructured with Winograd F(2,2) so their GEMM column
count drops 25% (total matmul cycles -14%):

Time axis is permuted host-side into 4 phases (t mod 4) so each chunk holds
4 contiguous phase blocks of J pair-columns. For the even chain
(y[4j], y[4j+2]) and odd chain (y[4j+1], y[4j+3]):
  m1 = (W0+W1) @ ph01[j]          (shared by both outputs of the pair)
  m2 = W0 @ (ph23[j-1] - ph01[j])
  m3 = W1 @ (ph23[j]   - ph01[j])
  y(ph01) = m1 + m2   (PE accumulates m2 in-place onto m1's PSUM bank)
  y(ph23) = m1 + m3   (m1 copied to SBUF between the groups; DVE adds)
All 1x1 GEMMs (cond/res/skip) are column-order invariant so they run on the
permuted layout unchanged; outputs are un-permuted host-side (free for HW).

Matmuls in bf16 (fp32 PSUM), N=512 per matmul, LDWEIGHTS hidden by FWL.
Startup: weight DMAs split across sync/gpsimd/scalar queues, wc split per
k-group, and 8 dummy matmuls warm the PE clock (HAM) during the DMA wait.
Chunks [256,512,1024,1024,1024,256]: small head for fast start, small tail
for a short drain.
"""

import os
import sys
from contextlib import ExitStack

import numpy as np

try:
    import concourse.bass as bass  # noqa: F401
except ImportError:  # pragma: no cover
    sys.path.insert(0, "/opt/trn_rl_repo")
    import concourse.bass as bass  # noqa: F401

import ml_dtypes  # noqa: E402
import concourse.tile as tile  # noqa: E402
from concourse import bacc, mybir  # noqa: E402
from concourse.bass_utils import run_bass_kernel_spmd  # noqa: E402

B, C, T = 8, 512, 4096
P, G = 128, 4          # SBUF partitions, channel groups (C = G*P)
Q = T // 4             # pair-columns per phase
CWS = [256, 512, 1024, 1024, 1024, 256]   # time-cols per chunk (mult of 4)
assert sum(CWS) == T
JS = [w // 4 for w in CWS]                # pair-cols per chunk
JOS = [0]
for _j in JS:
    JOS.append(JOS[-1] + _j)
NCH = len(CWS)

BF16 = mybir.dt.bfloat16
F32 = mybir.dt.float32
AF = mybir.ActivationFunctionType
ALU = mybir.AluOpType

_CACHE: dict = {}


def _build():
    nc = bacc.Bacc("TRN2", target_bir_lowering=False, debug=False, num_devices=B)

    def din(name, shape, dt):
        return nc.dram_tensor(name, shape, dt, kind="ExternalInput").ap()

    def dout(name, shape, dt):
        return nc.dram_tensor(name, shape, dt, kind="ExternalOutput").ap()

    # host pre-arranged to the device layout [p, g, f, q] = [c%P, c//P, t%4, t//4]
    # so chunk DMAs are 3D-mergeable strided reads
    x_d = din("x", [P, G, 4, Q], BF16)
    c_d = din("cond", [P, G, 4, Q], BF16)
    wc_d = din("wc", [P, G, G, P], BF16)
    wfs_d = din("wfs", [P, G, G, P], BF16)  # Wf0+Wf1
    wf0_d = din("wf0", [P, G, G, P], BF16)
    wf1_d = din("wf1", [P, G, G, P], BF16)
    wgs_d = din("wgs", [P, G, G, P], BF16)
    wg0_d = din("wg0", [P, G, G, P], BF16)
    wg1_d = din("wg1", [P, G, G, P], BF16)
    wr_d = din("wr", [P, G, G, P], BF16)
    ws_d = din("ws", [P, G, G, P], BF16)
    bias_d = din("bias", [P, 5 * G], F32)
    r_d = dout("res", [P, G, 4, Q], F32)
    s_d = dout("skip", [P, G, 4, Q], F32)

    x_r, c_r, r_r, s_r = x_d, c_d, r_d, s_d

    with tile.TileContext(nc) as tc, ExitStack() as ctx:
        const = ctx.enter_context(tc.tile_pool(name="const", bufs=1))
        cin = ctx.enter_context(tc.tile_pool(name="cin", bufs=2))
        xin = ctx.enter_context(tc.tile_pool(name="xin", bufs=2))
        xcp = ctx.enter_context(tc.tile_pool(name="xcp", bufs=3))
        dpl = ctx.enter_context(tc.tile_pool(name="dpl", bufs=4))
        m1p = ctx.enter_context(tc.tile_pool(name="m1p", bufs=4))
        fgp = ctx.enter_context(tc.tile_pool(name="fgp", bufs=10))
        hp = ctx.enter_context(tc.tile_pool(name="hp", bufs=3))
        rop = ctx.enter_context(tc.tile_pool(name="rop", bufs=4))
        sop = ctx.enter_context(tc.tile_pool(name="sop", bufs=4))
        psum = ctx.enter_context(tc.tile_pool(name="psum", bufs=8, space="PSUM"))

        from concourse.tile_rust import add_dep_helper

        # ---- PE warm-up: dummy matmuls issued while startup DMAs stream.
        # They depend only on a memset, so the PE is busy from the end of the
        # runtime preamble; the HAM clock-gate opens (~3.4us of activity)
        # before the first real matmul instead of 8us into the real stream.
        dummy = const.tile([P, 512], BF16)
        nc.vector.memset(dummy[:], 0.0)
        psD = psum.tile([P, 512], F32, space="PSUM", tag="ps")
        for _ in range(8):
            nc.tensor.matmul(psD, dummy[:, 0:128], dummy[:], start=True, stop=True)

        # ---- Startup DMA choreography.
        # sync (HWDGE): cond chunk0, wc split per k-group (so the first
        # accumulation chain can start as soon as k=0 lands), x chunk0, biases.
        # gpsimd (SWDGE): filter weights + wg taps, held behind wc so they
        # don't steal HBM bytes that gate the first matmul.
        # scalar (HWDGE): gate-sum + residual/skip weights (idle queue at start).
        ct0 = cin.tile([P, G, 4, JS[0]], BF16, tag="c")
        nc.sync.dma_start(ct0[:], c_r[:, :, :, 0:JS[0]])
        wc_k = []
        i_wc = None
        for k in range(G):
            t = const.tile([P, G, P], BF16, tag=f"wc{k}")
            i_wc = nc.sync.dma_start(t[:], wc_d[:, k])
            wc_k.append(t)
        xt0 = xin.tile([P, G, 4, JS[0]], BF16, tag="x")
        nc.sync.dma_start(xt0[:], x_r[:, :, :, 0:JS[0]])
        b_all = const.tile([P, 5 * G], F32)
        nc.sync.dma_start(b_all[:], bias_d)

        def gload(q, name, dram, dep=None):
            t = const.tile([P, G, G, P], BF16, tag=name)
            i = q.dma_start(t[:], dram)
            if dep is not None:
                add_dep_helper(i.ins, dep.ins, reason="prioritize wc bytes at startup")
            return t

        wfs_sb = gload(nc.gpsimd, "wfs", wfs_d, dep=i_wc)
        wf0_sb = gload(nc.gpsimd, "wf0", wf0_d)
        wf1_sb = gload(nc.gpsimd, "wf1", wf1_d)
        wg0_sb = gload(nc.gpsimd, "wg0", wg0_d)
        wg1_sb = gload(nc.gpsimd, "wg1", wg1_d)
        wgs_sb = gload(nc.scalar, "wgs", wgs_d)
        wr_sb = gload(nc.scalar, "wr", wr_d)
        ws_sb = gload(nc.scalar, "ws", ws_d)
        wsum = (wfs_sb, wgs_sb)
        wtap0 = (wf0_sb, wg0_sb)
        wtap1 = (wf1_sb, wg1_sb)

        pre_c = {0: ct0}
        pre_x = {0: xt0}
        _bidx = {"bc": 0, "bf": 1, "bg": 2, "br": 3, "bs": 4}

        def bias_ap(name, m):
            return b_all[:, _bidx[name] * G + m:_bidx[name] * G + m + 1]

        xc_t: dict = {}
        h_t: dict = {}
        for it in range(NCH + 2):
            c0, c1, c2 = it, it - 1, it - 2

            # ---- stage 1: condition injection (chunk c0) ----
            # xc[:, m, ph, 1+j] = x + Wc@cond + bc at t = 4*(jo+j)+ph;
            # col 0 of each phase is the halo (previous chunk's last pair).
            if c0 < NCH:
                J, jo = JS[c0], JOS[c0]
                if c0 in pre_c:
                    ct, xt = pre_c.pop(c0), pre_x.pop(c0)
                else:
                    ct = cin.tile([P, G, 4, J], BF16, tag="c")
                    nc.sync.dma_start(ct[:], c_r[:, :, :, jo:jo + J])
                    xt = xin.tile([P, G, 4, J], BF16, tag="x")
                    nc.sync.dma_start(xt[:], x_r[:, :, :, jo:jo + J])
                xc = xcp.tile([P, G, 4, J + 1], BF16, tag="xc")
                if c0 == 0:
                    nc.vector.memset(xc[:, :, :, 0:1], 0.0)
                else:
                    Jp = JS[c0 - 1]
                    nc.vector.tensor_copy(xc[:, :, :, 0:1],
                                          xc_t[c0 - 1][:, :, :, Jp:Jp + 1])
                for m in range(G):
                    for h2 in range(2):
                        ps = psum.tile([P, 2, J], F32, space="PSUM", tag="ps")
                        for k in range(G):
                            nc.tensor.matmul(ps, wc_k[k][:, m, :],
                                             ct[:, k, 2 * h2:2 * h2 + 2, :],
                                             start=(k == 0), stop=(k == G - 1))
                        nc.vector.scalar_tensor_tensor(
                            xc[:, m, 2 * h2:2 * h2 + 2, 1:J + 1], ps,
                            bias_ap("bc", m), xt[:, m, 2 * h2:2 * h2 + 2, :],
                            ALU.add, ALU.add)
                xc_t[c0] = xc

            # ---- stage 2: Winograd dilated conv + gated activation (c1) ----
            if 0 <= c1 < NCH:
                J = JS[c1]
                xc = xc_t[c1]
                d2 = dpl.tile([P, G, 2, J], BF16, tag="d2")
                nc.vector.tensor_sub(d2[:], xc[:, :, 2:4, 0:J],
                                     xc[:, :, 0:2, 1:J + 1])
                d3 = dpl.tile([P, G, 2, J], BF16, tag="d3")
                nc.vector.tensor_sub(d3[:], xc[:, :, 2:4, 1:J + 1],
                                     xc[:, :, 0:2, 1:J + 1])
                h = hp.tile([P, G, 4, J], BF16, tag="h")
                for m in range(G):
                    pA, pB, m1sb = [], [], []
                    # m1 = Wsum @ ph01 for both convs
                    for cv in range(2):
                        ps = psum.tile([P, 2, J], F32, space="PSUM", tag="ps")
                        for k in range(G):
                            nc.tensor.matmul(ps, wsum[cv][:, k, m, :],
                                             xc[:, k, 0:2, 1:J + 1],
                                             start=(k == 0), stop=(k == G - 1))
                        pA.append(ps)
                    # snapshot m1 (needed again for the ph23 outputs)
                    for cv in range(2):
                        t = m1p.tile([P, 2, J], BF16, tag="m1")
                        nc.vector.tensor_copy(t[:], pA[cv])
                        m1sb.append(t)
                    # m3 = Wtap1 @ d3 (fresh banks) -- gives the copies slack
                    for cv in range(2):
                        ps = psum.tile([P, 2, J], F32, space="PSUM", tag="ps")
                        for k in range(G):
                            nc.tensor.matmul(ps, wtap1[cv][:, k, m, :], d3[:, k],
                                             start=(k == 0), stop=(k == G - 1))
                        pB.append(ps)
                    # m2 = Wtap0 @ d2 accumulated in-place onto m1 -> y(ph01)
                    for cv in range(2):
                        for k in range(G):
                            nc.tensor.matmul(pA[cv], wtap0[cv][:, k, m, :],
                                             d2[:, k], start=False,
                                             stop=(k == G - 1),
                                             skip_group_check=True)
                    fg01, fg23 = [], []
                    for cv, afn, bn in ((0, AF.Tanh, "bf"),
                                        (1, AF.Sigmoid, "bg")):
                        t01 = fgp.tile([P, 2, J], BF16, tag="fg")
                        nc.scalar.activation(t01, pA[cv], afn,
                                             bias=bias_ap(bn, m))
                        p23 = fgp.tile([P, 2, J], BF16, tag="fg")
                        nc.vector.tensor_add(p23, pB[cv], m1sb[cv])
                        t23 = fgp.tile([P, 2, J], BF16, tag="fg")
                        nc.scalar.activation(t23, p23, afn,
                                             bias=bias_ap(bn, m))
                        fg01.append(t01)
                        fg23.append(t23)
                    nc.vector.tensor_mul(h[:, m, 0:2, :], fg01[0], fg01[1])
                    nc.vector.tensor_mul(h[:, m, 2:4, :], fg23[0], fg23[1])
                h_t[c1] = h

            # ---- stage 3: residual + skip projections (chunk c2) ----
            if 0 <= c2 < NCH:
                J, jo = JS[c2], JOS[c2]
                h = h_t.pop(c2)
                xc = xc_t.pop(c2)
                for m in range(G):
                    rt = rop.tile([P, 4, J], F32, tag="r")
                    st = sop.tile([P, 4, J], F32, tag="s")
                    for h2 in range(2):
                        sl = slice(2 * h2, 2 * h2 + 2)
                        pr = psum.tile([P, 2, J], F32, space="PSUM", tag="ps")
                        for k in range(G):
                            nc.tensor.matmul(pr, wr_sb[:, k, m, :],
                                             h[:, k, sl, :],
                                             start=(k == 0), stop=(k == G - 1))
                        nc.vector.scalar_tensor_tensor(
                            rt[:, sl, :], pr, bias_ap("br", m),
                            xc[:, m, sl, 1:J + 1], ALU.add, ALU.add)
                        pk = psum.tile([P, 2, J], F32, space="PSUM", tag="ps")
                        for k in range(G):
                            nc.tensor.matmul(pk, ws_sb[:, k, m, :],
                                             h[:, k, sl, :],
                                             start=(k == 0), stop=(k == G - 1))
                        nc.scalar.activation(st[:, sl, :], pk, AF.Identity,
                                             bias=bias_ap("bs", m))
                    nc.scalar.dma_start(r_r[:, m, :, jo:jo + J], rt[:])
                    nc.gpsimd.dma_start(s_r[:, m, :, jo:jo + J], st[:])

    nc.compile()
    return nc


def _get_nc():
    if "nc" not in _CACHE:
        _CACHE["nc"] = _build()
    return _CACHE["nc"]


def _wT1(w):
    # [Cout, Cin] -> lhsT layout [P(cin%P), G(cin//P), G(cout//P), P(cout%P)]
    return np.ascontiguousarray(
        np.asarray(w, dtype=np.float32).T.reshape(G, P, G, P)
        .transpose(1, 0, 2, 3).astype(ml_dtypes.bfloat16))


def _bias(b):
    return np.ascontiguousarray(np.asarray(b).reshape(G, P).T.astype(np.float32))


def _phase_perm(a):
    # [B, C, T] -> [B, P, G, 4, Q]: [b, c%P, c//P, t%4, t//4], bf16
    return np.ascontiguousarray(
        np.asarray(a).reshape(B, G, P, Q, 4).transpose(0, 2, 1, 4, 3)
        .astype(ml_dtypes.bfloat16))


def kernel(x, condition, wf, bf, wg, bg, wr, br, ws, bs, wc, bc):
    nc = _get_nc()
    x_p = _phase_perm(x)
    c_p = _phase_perm(condition)
    wf = np.asarray(wf, dtype=np.float32)
    wg = np.asarray(wg, dtype=np.float32)
    shared = {
        "wc": _wT1(np.asarray(wc)[:, :, 0]),
        "wfs": _wT1(wf[:, :, 0] + wf[:, :, 1]),
        "wf0": _wT1(wf[:, :, 0]),
        "wf1": _wT1(wf[:, :, 1]),
        "wgs": _wT1(wg[:, :, 0] + wg[:, :, 1]),
        "wg0": _wT1(wg[:, :, 0]),
        "wg1": _wT1(wg[:, :, 1]),
        "wr": _wT1(np.asarray(wr)[:, :, 0]),
        "ws": _wT1(np.asarray(ws)[:, :, 0]),
        "bias": np.ascontiguousarray(np.concatenate(
            [_bias(b) for b in (bc, bf, bg, br, bs)], axis=1)),
    }
    in_maps = [
        {"x": np.ascontiguousarray(x_p[i]), "cond": np.ascontiguousarray(c_p[i]),
         **shared}
        for i in range(B)
    ]
    res = run_bass_kernel_spmd(
        nc, in_maps, list(range(B)),
        trace=bool(os.environ.get("CC_KERNEL_TRACE")))
    _CACHE["last_results"] = res

    def unperm(name):
        # [P, G, 4, Q] -> [C, T]
        return np.stack([
            res.results[i][name].transpose(1, 0, 3, 2).reshape(C, T)
            for i in range(B)
        ])

    return unperm("res"), unperm("skip")


# revision 15
# speedup vs baseline: 1.1068x; 1.0493x over previous
"""WaveNet-style gated residual block (AdvancedSkipResidualBlock) on 8 TRN2 NeuronCores.

Strategy: data-parallel over batch B=8 -> one batch element per NeuronCore.
Per core the block is 7 GEMMs of [512,512] x [512,T]; the two dilated causal
convs (K=2, d=2) are restructured with Winograd F(2,2) so their GEMM column
count drops 25% (total matmul cycles -14%):

Time axis is permuted host-side into 4 phases (t mod 4) so each chunk holds
4 contiguous phase blocks of J pair-columns. For the even chain
(y[4j], y[4j+2]) and odd chain (y[4j+1], y[4j+3]):
  m1 = (W0+W1) @ ph01[j]          (shared by both outputs of the pair)
  m2 = W0 @ (ph23[j-1] - ph01[j])
  m3 = W1 @ (ph23[j]   - ph01[j])
  y(ph01) = m1 + m2   (PE accumulates m2 in-place onto m1's PSUM bank)
  y(ph23) = m1 + m3   (m1 copied to SBUF between the groups; DVE adds)
All 1x1 GEMMs (cond/res/skip) are column-order invariant so they run on the
permuted layout unchanged; outputs are un-permuted host-side (free for HW).

Matmuls in bf16 (fp32 PSUM), N=512 per matmul, LDWEIGHTS hidden by FWL.
Startup: weight DMAs split across sync/gpsimd/scalar queues, wc split per
k-group, and 8 dummy matmuls warm the PE clock (HAM) during the DMA wait.
Chunks [256,512,1024,1024,1024,256]: small head for fast start, small tail
for a short drain.
"""

import os
import sys
from contextlib import ExitStack

import numpy as np

try:
    import concourse.bass as bass  # noqa: F401
except ImportError:  # pragma: no cover
    sys.path.insert(0, "/opt/trn_rl_repo")
    import concourse.bass as bass  # noqa: F401

import ml_dtypes  # noqa: E402
import concourse.tile as tile  # noqa: E402
from concourse import bacc, mybir  # noqa: E402
from concourse.bass_utils import run_bass_kernel_spmd  # noqa: E402

B, C, T = 8, 512, 4096
P, G = 128, 4          # SBUF partitions, channel groups (C = G*P)
Q = T // 4             # pair-columns per phase
CWS = [256, 512, 1024, 1024, 1024, 256]   # time-cols per chunk (mult of 4)
assert sum(CWS) == T
JS = [w // 4 for w in CWS]                # pair-cols per chunk
JOS = [0]
for _j in JS:
    JOS.append(JOS[-1] + _j)
NCH = len(CWS)

BF16 = mybir.dt.bfloat16
F32 = mybir.dt.float32
AF = mybir.ActivationFunctionType
ALU = mybir.AluOpType

_CACHE: dict = {}


def _build():
    nc = bacc.Bacc("TRN2", target_bir_lowering=False, debug=False, num_devices=B)

    def din(name, shape, dt):
        return nc.dram_tensor(name, shape, dt, kind="ExternalInput").ap()

    def dout(name, shape, dt):
        return nc.dram_tensor(name, shape, dt, kind="ExternalOutput").ap()

    # host pre-arranged per-chunk-blocked: for chunk c the cols
    # [16*JOS[c] : 16*JOS[c]+16*J] hold the [G, 4, J] block (g, t%4, pair)
    # of partition p. Chunk DMAs are then 128 segments of 16*J*2B.
    x_d = din("x", [P, 4 * G * Q], BF16)
    c_d = din("cond", [P, 4 * G * Q], BF16)
    wc_d = din("wc", [P, G, G, P], BF16)
    wfs_d = din("wfs", [P, G, G, P], BF16)  # Wf0+Wf1
    wf0_d = din("wf0", [P, G, G, P], BF16)
    wf1_d = din("wf1", [P, G, G, P], BF16)
    wgs_d = din("wgs", [P, G, G, P], BF16)
    wg0_d = din("wg0", [P, G, G, P], BF16)
    wg1_d = din("wg1", [P, G, G, P], BF16)
    wr_d = din("wr", [P, G, G, P], BF16)
    ws_d = din("ws", [P, G, G, P], BF16)
    bias_d = din("bias", [P, 5 * G], F32)
    r_d = dout("res", [P, 4 * G * Q], F32)
    s_d = dout("skip", [P, 4 * G * Q], F32)

    x_r, c_r, r_r, s_r = x_d, c_d, r_d, s_d

    with tile.TileContext(nc) as tc, ExitStack() as ctx:
        const = ctx.enter_context(tc.tile_pool(name="const", bufs=1))
        cin = ctx.enter_context(tc.tile_pool(name="cin", bufs=2))
        xin = ctx.enter_context(tc.tile_pool(name="xin", bufs=2))
        xcp = ctx.enter_context(tc.tile_pool(name="xcp", bufs=3))
        dpl = ctx.enter_context(tc.tile_pool(name="dpl", bufs=4))
        m1p = ctx.enter_context(tc.tile_pool(name="m1p", bufs=4))
        fgp = ctx.enter_context(tc.tile_pool(name="fgp", bufs=10))
        hp = ctx.enter_context(tc.tile_pool(name="hp", bufs=3))
        rop = ctx.enter_context(tc.tile_pool(name="rop", bufs=4))
        sop = ctx.enter_context(tc.tile_pool(name="sop", bufs=4))
        psum = ctx.enter_context(tc.tile_pool(name="psum", bufs=8, space="PSUM"))

        from concourse.tile_rust import add_dep_helper

        # ---- PE warm-up: dummy matmuls issued while startup DMAs stream.
        # They depend only on a memset, so the PE is busy from the end of the
        # runtime preamble; the HAM clock-gate opens (~3.4us of activity)
        # before the first real matmul instead of 8us into the real stream.
        dummy = const.tile([P, 512], BF16)
        nc.vector.memset(dummy[:], 0.0)
        psD = psum.tile([P, 512], F32, space="PSUM", tag="ps")
        for _ in range(8):
            nc.tensor.matmul(psD, dummy[:, 0:128], dummy[:], start=True, stop=True)

        # ---- Startup DMA choreography.
        # sync (HWDGE): cond chunk0, wc split per k-group (so the first
        # accumulation chain can start as soon as k=0 lands), x chunk0, biases.
        # gpsimd (SWDGE): filter weights + wg taps, held behind wc so they
        # don't steal HBM bytes that gate the first matmul.
        # scalar (HWDGE): gate-sum + residual/skip weights (idle queue at start).
        ct0 = cin.tile([P, G, 4, JS[0]], BF16, tag="c")
        nc.sync.dma_start(ct0[:], c_r[:, 0:16 * JS[0]])
        wc_k = []
        i_wc = None
        for k in range(G):
            t = const.tile([P, G, P], BF16, tag=f"wc{k}")
            i_wc = nc.sync.dma_start(t[:], wc_d[:, k])
            wc_k.append(t)
        xt0 = xin.tile([P, G, 4, JS[0]], BF16, tag="x")
        nc.sync.dma_start(xt0[:], x_r[:, 0:16 * JS[0]])
        b_all = const.tile([P, 5 * G], F32)
        nc.sync.dma_start(b_all[:], bias_d)

        def gload(q, name, dram, dep=None):
            t = const.tile([P, G, G, P], BF16, tag=name)
            i = q.dma_start(t[:], dram)
            if dep is not None:
                add_dep_helper(i.ins, dep.ins, reason="prioritize wc bytes at startup")
            return t

        wfs_sb = gload(nc.gpsimd, "wfs", wfs_d, dep=i_wc)
        wf0_sb = gload(nc.gpsimd, "wf0", wf0_d)
        wf1_sb = gload(nc.gpsimd, "wf1", wf1_d)
        wg0_sb = gload(nc.gpsimd, "wg0", wg0_d)
        wg1_sb = gload(nc.gpsimd, "wg1", wg1_d)
        wgs_sb = gload(nc.scalar, "wgs", wgs_d)
        wr_sb = gload(nc.scalar, "wr", wr_d)
        ws_sb = gload(nc.scalar, "ws", ws_d)
        wsum = (wfs_sb, wgs_sb)
        wtap0 = (wf0_sb, wg0_sb)
        wtap1 = (wf1_sb, wg1_sb)

        pre_c = {0: ct0}
        pre_x = {0: xt0}
        _bidx = {"bc": 0, "bf": 1, "bg": 2, "br": 3, "bs": 4}

        def bias_ap(name, m):
            return b_all[:, _bidx[name] * G + m:_bidx[name] * G + m + 1]

        xc_t: dict = {}
        h_t: dict = {}
        for it in range(NCH + 2):
            c0, c1, c2 = it, it - 1, it - 2

            # ---- stage 1: condition injection (chunk c0) ----
            # xc[:, m, ph, 1+j] = x + Wc@cond + bc at t = 4*(jo+j)+ph;
            # col 0 of each phase is the halo (previous chunk's last pair).
            if c0 < NCH:
                J, jo = JS[c0], JOS[c0]
                if c0 in pre_c:
                    ct, xt = pre_c.pop(c0), pre_x.pop(c0)
                else:
                    ct = cin.tile([P, G, 4, J], BF16, tag="c")
                    nc.sync.dma_start(ct[:], c_r[:, 16 * jo:16 * (jo + J)])
                    xt = xin.tile([P, G, 4, J], BF16, tag="x")
                    nc.sync.dma_start(xt[:], x_r[:, 16 * jo:16 * (jo + J)])
                xc = xcp.tile([P, G, 4, J + 1], BF16, tag="xc")
                if c0 == 0:
                    nc.vector.memset(xc[:, :, :, 0:1], 0.0)
                else:
                    Jp = JS[c0 - 1]
                    nc.vector.tensor_copy(xc[:, :, :, 0:1],
                                          xc_t[c0 - 1][:, :, :, Jp:Jp + 1])
                for m in range(G):
                    for h2 in range(2):
                        ps = psum.tile([P, 2, J], F32, space="PSUM", tag="ps")
                        for k in range(G):
                            nc.tensor.matmul(ps, wc_k[k][:, m, :],
                                             ct[:, k, 2 * h2:2 * h2 + 2, :],
                                             start=(k == 0), stop=(k == G - 1))
                        nc.vector.scalar_tensor_tensor(
                            xc[:, m, 2 * h2:2 * h2 + 2, 1:J + 1], ps,
                            bias_ap("bc", m), xt[:, m, 2 * h2:2 * h2 + 2, :],
                            ALU.add, ALU.add)
                xc_t[c0] = xc

            # ---- stage 2: Winograd dilated conv + gated activation (c1) ----
            if 0 <= c1 < NCH:
                J = JS[c1]
                xc = xc_t[c1]
                d2 = dpl.tile([P, G, 2, J], BF16, tag="d2")
                nc.vector.tensor_sub(d2[:], xc[:, :, 2:4, 0:J],
                                     xc[:, :, 0:2, 1:J + 1])
                d3 = dpl.tile([P, G, 2, J], BF16, tag="d3")
                nc.vector.tensor_sub(d3[:], xc[:, :, 2:4, 1:J + 1],
                                     xc[:, :, 0:2, 1:J + 1])
                h = hp.tile([P, G, 4, J], BF16, tag="h")
                for m in range(G):
                    pA, pB, m1sb = [], [], []
                    # m1 = Wsum @ ph01 for both convs
                    for cv in range(2):
                        ps = psum.tile([P, 2, J], F32, space="PSUM", tag="ps")
                        for k in range(G):
                            nc.tensor.matmul(ps, wsum[cv][:, k, m, :],
                                             xc[:, k, 0:2, 1:J + 1],
                                             start=(k == 0), stop=(k == G - 1))
                        pA.append(ps)
                    # snapshot m1 (needed again for the ph23 outputs)
                    for cv in range(2):
                        t = m1p.tile([P, 2, J], BF16, tag="m1")
                        nc.vector.tensor_copy(t[:], pA[cv])
                        m1sb.append(t)
                    # m3 = Wtap1 @ d3 (fresh banks) -- gives the copies slack
                    for cv in range(2):
                        ps = psum.tile([P, 2, J], F32, space="PSUM", tag="ps")
                        for k in range(G):
                            nc.tensor.matmul(ps, wtap1[cv][:, k, m, :], d3[:, k],
                                             start=(k == 0), stop=(k == G - 1))
                        pB.append(ps)
                    # m2 = Wtap0 @ d2 accumulated in-place onto m1 -> y(ph01)
                    for cv in range(2):
                        for k in range(G):
                            nc.tensor.matmul(pA[cv], wtap0[cv][:, k, m, :],
                                             d2[:, k], start=False,
                                             stop=(k == G - 1),
                                             skip_group_check=True)
                    fg01, fg23 = [], []
                    for cv, afn, bn in ((0, AF.Tanh, "bf"),
                                        (1, AF.Sigmoid, "bg")):
                        t01 = fgp.tile([P, 2, J], BF16, tag="fg")
                        nc.scalar.activation(t01, pA[cv], afn,
                                             bias=bias_ap(bn, m))
                        p23 = fgp.tile([P, 2, J], BF16, tag="fg")
                        nc.vector.tensor_add(p23, pB[cv], m1sb[cv])
                        t23 = fgp.tile([P, 2, J], BF16, tag="fg")
                        nc.scalar.activation(t23, p23, afn,
                                             bias=bias_ap(bn, m))
                        fg01.append(t01)
                        fg23.append(t23)
                    nc.vector.tensor_mul(h[:, m, 0:2, :], fg01[0], fg01[1])
                    nc.vector.tensor_mul(h[:, m, 2:4, :], fg23[0], fg23[1])
                h_t[c1] = h

            # ---- stage 3: residual + skip projections (chunk c2) ----
            if 0 <= c2 < NCH:
                J, jo = JS[c2], JOS[c2]
                h = h_t.pop(c2)
                xc = xc_t.pop(c2)
                for m in range(G):
                    rt = rop.tile([P, 4, J], F32, tag="r")
                    st = sop.tile([P, 4, J], F32, tag="s")
                    for h2 in range(2):
                        sl = slice(2 * h2, 2 * h2 + 2)
                        pr = psum.tile([P, 2, J], F32, space="PSUM", tag="ps")
                        for k in range(G):
                            nc.tensor.matmul(pr, wr_sb[:, k, m, :],
                                             h[:, k, sl, :],
                                             start=(k == 0), stop=(k == G - 1))
                        nc.vector.scalar_tensor_tensor(
                            rt[:, sl, :], pr, bias_ap("br", m),
                            xc[:, m, sl, 1:J + 1], ALU.add, ALU.add)
                        pk = psum.tile([P, 2, J], F32, space="PSUM", tag="ps")
                        for k in range(G):
                            nc.tensor.matmul(pk, ws_sb[:, k, m, :],
                                             h[:, k, sl, :],
                                             start=(k == 0), stop=(k == G - 1))
                        nc.scalar.activation(st[:, sl, :], pk, AF.Identity,
                                             bias=bias_ap("bs", m))
                    off = 16 * jo + m * 4 * J
                    nc.scalar.dma_start(r_r[:, off:off + 4 * J], rt[:])
                    nc.gpsimd.dma_start(s_r[:, off:off + 4 * J], st[:])

    nc.compile()
    return nc


def _get_nc():
    if "nc" not in _CACHE:
        _CACHE["nc"] = _build()
    return _CACHE["nc"]


def _wT1(w):
    # [Cout, Cin] -> lhsT layout [P(cin%P), G(cin//P), G(cout//P), P(cout%P)]
    return np.ascontiguousarray(
        np.asarray(w, dtype=np.float32).T.reshape(G, P, G, P)
        .transpose(1, 0, 2, 3).astype(ml_dtypes.bfloat16))


def _bias(b):
    return np.ascontiguousarray(np.asarray(b).reshape(G, P).T.astype(np.float32))


def _phase_perm(a):
    # [B, C, T] -> [B, P, 16Q] per-chunk-blocked: chunk c holds the
    # [G, 4, J_c] block (g, t%4, pair) flattened, chunks concatenated.
    ap = (np.asarray(a).reshape(B, G, P, Q, 4).transpose(0, 2, 1, 4, 3)
          .astype(ml_dtypes.bfloat16))          # [B, P, G, 4, Q]
    blocks = [
        np.ascontiguousarray(ap[:, :, :, :, JOS[c]:JOS[c] + JS[c]])
        .reshape(B, P, 16 * JS[c])
        for c in range(NCH)
    ]
    return np.ascontiguousarray(np.concatenate(blocks, axis=2))


def kernel(x, condition, wf, bf, wg, bg, wr, br, ws, bs, wc, bc):
    nc = _get_nc()
    x_p = _phase_perm(x)
    c_p = _phase_perm(condition)
    wf = np.asarray(wf, dtype=np.float32)
    wg = np.asarray(wg, dtype=np.float32)
    shared = {
        "wc": _wT1(np.asarray(wc)[:, :, 0]),
        "wfs": _wT1(wf[:, :, 0] + wf[:, :, 1]),
        "wf0": _wT1(wf[:, :, 0]),
        "wf1": _wT1(wf[:, :, 1]),
        "wgs": _wT1(wg[:, :, 0] + wg[:, :, 1]),
        "wg0": _wT1(wg[:, :, 0]),
        "wg1": _wT1(wg[:, :, 1]),
        "wr": _wT1(np.asarray(wr)[:, :, 0]),
        "ws": _wT1(np.asarray(ws)[:, :, 0]),
        "bias": np.ascontiguousarray(np.concatenate(
            [_bias(b) for b in (bc, bf, bg, br, bs)], axis=1)),
    }
    in_maps = [
        {"x": np.ascontiguousarray(x_p[i]), "cond": np.ascontiguousarray(c_p[i]),
         **shared}
        for i in range(B)
    ]
    res = run_bass_kernel_spmd(
        nc, in_maps, list(range(B)),
        trace=bool(os.environ.get("CC_KERNEL_TRACE")))
    _CACHE["last_results"] = res

    def unperm(name):
        # per-chunk-blocked [P, 16Q] -> [C, T]
        out = np.empty((B, P, G, 4, Q), dtype=np.float32)
        for i in range(B):
            flat = res.results[i][name]
            for c in range(NCH):
                jo, J = JOS[c], JS[c]
                out[i, :, :, :, jo:jo + J] = (
                    flat[:, 16 * jo:16 * (jo + J)].reshape(P, G, 4, J))
        return np.ascontiguousarray(
            out.transpose(0, 2, 1, 4, 3).reshape(B, C, T))

    return unperm("res"), unperm("skip")
